# revision 1
# baseline (speedup 1.0000x reference)
"""Trainium2 Bass kernel: 3x3 SAME conv (stride 1), NCHW fp32.

Problem: image [32, 64, 112, 112] * weight [64, 64, 3, 3] + bias [64]
Sharding: data-parallel over batch across 8 NeuronCores (4 images each).

Per-core strategy:
  - Image stored in SBUF padded to 114x114, split into two vertical halves
    (58 padded rows each, 2 overlap/halo rows) living on partition ranges
    0-63 (upper half) and 64-127 (lower half); partition p = 64*s + cin.
  - Conv = 9 accumulating matmuls (one per filter tap) into PSUM. With the
    W-padded flat layout, tap (kh, kw) is a pure shift of the rhs AP:
    rhs = x[cin, (4*rb+kh)*114 + kw : +456]. Output tile = 4 output rows
    (456 = 4*114 PSUM columns, 2 garbage columns per row dropped on drain).
  - The 128x128 PE array is quadrant-tiled: K=64 (cin) x M=64 (cout) uses
    one (row-group, col-group) quadrant, so 4 matmuls (2 image halves x 2
    output tiles) run concurrently, selected implicitly by the base
    partitions of lhsT/rhs (row) and the PSUM slice (col).
  - Operands are bf16 (converted on host): full-rate 1 col/cycle matmuls
    with legal quadrant placement (fp32r/fp32 matmuls fail the s3d3
    dst-partition ISA check for off-diagonal quadrants), fp32 PSUM
    accumulation, and half the input HBM traffic. Measured rel err vs
    the fp32 reference: ~2.4e-3.
  - Drain: DVE tensor_scalar_add (fused +bias) PSUM -> SBUF staging,
    dropping the garbage columns, then one contiguous 128-partition DMA
    per staging tile back to HBM (each channel's 2x4 output rows are one
    3584-byte contiguous run).
"""

import numpy as np

import concourse.bass as bass
import concourse.mybir as mybir
import concourse.tile as tile
from concourse import bacc, bass_utils

N_CORES = 8
IMGS = 4  # images per core
CIN = 64
COUT = 64
H = 112
W = 112
HP = H + 2  # 114
WP = W + 2  # 114
HALF_OUT_ROWS = 56  # output rows per half
HALF_ROWS = HALF_OUT_ROWS + 2  # 58 local padded rows per half
F = 1 + HALF_ROWS * WP + 1  # 6614 floats per partition (lead+trail pad cells)
RB = 14  # row blocks per half, 4 output rows each
NMM = 4 * WP  # 456 matmul free size
NOUT = 4 * W  # 448 valid outputs per tile per channel

F32 = mybir.dt.float32
F32R = mybir.dt.float32r
BF16 = mybir.dt.bfloat16


def _ap(ap_obj, offset, dims):
    """Manual AP on the same tensor handle; dims = [[step, count], ...]."""
    return bass.AP(tensor=ap_obj.tensor, offset=offset, ap=dims)


def build_nc(n_imgs=IMGS, mm_dtype=BF16):
    nc = bacc.Bacc(
        "TRN2",
        target_bir_lowering=False,
        debug=False,
        num_devices=N_CORES,
    )
    # image/weight are bf16 end-to-end (host-converted)
    img_d = nc.dram_tensor("image_pad", [n_imgs, CIN, HP, WP], mm_dtype, kind="ExternalInput")
    wt_d = nc.dram_tensor("weight2", [128, 9 * COUT], mm_dtype, kind="ExternalInput")
    bias_d = nc.dram_tensor("bias2", [128, 1], F32, kind="ExternalInput")
    out_d = nc.dram_tensor("out", [n_imgs, COUT, H, W], F32, kind="ExternalOutput")

    img_ap = img_d.ap()
    out_ap = out_d.ap()

    with tile.TileContext(nc) as tc:
        with (
            tc.tile_pool(name="img", bufs=2) as img_pool,
            tc.tile_pool(name="wt", bufs=1) as wt_pool,
            tc.tile_pool(name="bias", bufs=1) as bias_pool,
            tc.tile_pool(name="stage", bufs=6) as stage_pool,
            tc.tile_pool(name="psum", bufs=4, space="PSUM") as psum_pool,
        ):
            wt_t = wt_pool.tile([128, 9 * COUT], mm_dtype)
            nc.sync.dma_start(wt_t[:], wt_d.ap()[:])
            bias_t = bias_pool.tile([128, 1], F32)
            nc.sync.dma_start(bias_t[:], bias_d.ap()[:])

            for n in range(n_imgs):
                img_t = img_pool.tile([128, F], mm_dtype)
                # lead/trail pad cells stay uninitialized: they only feed
                # the dropped garbage columns (c_out 0/113) of edge tiles.
                # one 128-partition DMA: partition 64*s + c <- padded rows
                # [56*s, 56*s + 58) of channel c, 58*114 floats contiguous
                src = _ap(
                    img_ap,
                    n * CIN * HP * WP,
                    [[HALF_OUT_ROWS * WP, 2], [HP * WP, CIN], [1, HALF_ROWS * WP]],
                )
                nc.sync.dma_start(img_t[:, 1 : 1 + HALF_ROWS * WP], src)

                for q in range(RB // 2):
                    rb0, rb1 = 2 * q, 2 * q + 1
                    psum_a = psum_pool.tile([128, NMM], F32)
                    psum_b = psum_pool.tile([128, NMM], F32)
                    for tap in range(9):
                        kh, kw = divmod(tap, 3)
                        st, sp = tap == 0, tap == 8
                        off0 = (4 * rb0 + kh) * WP + kw
                        off1 = (4 * rb1 + kh) * WP + kw
                        w_lo = wt_t[0:64, tap * 64 : (tap + 1) * 64].bitcast(mm_dtype)
                        w_hi = wt_t[64:128, tap * 64 : (tap + 1) * 64].bitcast(mm_dtype)
                        x00 = img_t[0:64, off0 : off0 + NMM].bitcast(mm_dtype)
                        x10 = img_t[64:128, off0 : off0 + NMM].bitcast(mm_dtype)
                        x01 = img_t[0:64, off1 : off1 + NMM].bitcast(mm_dtype)
                        x11 = img_t[64:128, off1 : off1 + NMM].bitcast(mm_dtype)
                        # quadrants (row_grp, col_grp) from base partitions:
                        # psum_a = half0 x {rb0, rb1} via (0,0),(0,64);
                        # psum_b = half1 x {rb0, rb1} via (64,0),(64,64).
                        # skip_group_check: the sim's psum-group checker
                        # mis-anchors marks for base-partition-64 slices;
                        # per-element has_written on HW handles this fine.
                        nc.tensor.matmul(
                            psum_a[0:64, :], w_lo, x00,
                            start=st, stop=sp, skip_group_check=True)
                        nc.tensor.matmul(
                            psum_a[64:128, :], w_lo, x01,
                            start=st, stop=sp, skip_group_check=True)
                        nc.tensor.matmul(
                            psum_b[0:64, :], w_hi, x10,
                            start=st, stop=sp, skip_group_check=True)
                        nc.tensor.matmul(
                            psum_b[64:128, :], w_hi, x11,
                            start=st, stop=sp, skip_group_check=True)

                    # drain + bias: psum[:, 4x(1..113)] -> stage[:, 448].
                    # psum_a partitions = [rb0|rb1] of half0; psum_b same of
                    # half1 -> one contiguous-run DMA each (8 rows per chan).
                    for h, ps in ((0, psum_a), (1, psum_b)):
                        stg = stage_pool.tile([128, NOUT], F32)
                        src_ps = ps[:].rearrange("p (r c) -> p r c", r=4)[:, :, 1 : 1 + W]
                        dst_st = stg[:].rearrange("p (r c) -> p r c", r=4)
                        nc.vector.tensor_scalar_add(dst_st, src_ps, bias_t[:])
                        base = n * COUT * H * W + (h * HALF_OUT_ROWS + 4 * rb0) * W
                        dst = _ap(
                            out_ap,
                            base,
                            [[4 * W, 2], [H * W, COUT], [1, NOUT]],
                        )
                        nc.sync.dma_start(dst, stg[:])

    nc.compile()
    return nc


_NC_CACHE = {}


def _get_nc(n_imgs=IMGS):
    if n_imgs not in _NC_CACHE:
        _NC_CACHE[n_imgs] = build_nc(n_imgs)
    return _NC_CACHE[n_imgs]


def _prep_inputs(image, weight, bias):
    import ml_dtypes

    image = np.asarray(image, dtype=np.float32)
    weight = np.asarray(weight, dtype=np.float32)
    bias = np.asarray(bias, dtype=np.float32).astype(np.float32)
    n = image.shape[0]
    bf16 = ml_dtypes.bfloat16
    img_pad = np.zeros((n, CIN, HP, WP), bf16)
    img_pad[:, :, 1 : 1 + H, 1 : 1 + W] = image.astype(bf16)
    # lhsT layout per tap: [cin, cout], taps flattened; duplicated on both
    # partition halves for the two PE row groups.
    wt = np.ascontiguousarray(
        weight.transpose(1, 2, 3, 0).reshape(CIN, 9 * COUT)
    ).astype(bf16)
    wt2 = np.concatenate([wt, wt], axis=0)
    b2 = np.concatenate([bias, bias]).reshape(128, 1)
    return img_pad, wt2, b2


def run_cores(image, weight, bias, trace=False, **kw):
    """Shard over 8 cores, run, return (full_output, BassKernelResults)."""
    img_pad, wt2, b2 = _prep_inputs(image, weight, bias)
    n = img_pad.shape[0]
    per = n // N_CORES
    assert per * N_CORES == n
    nc = _get_nc(per)
    in_maps = [
        {
            "image_pad": np.ascontiguousarray(img_pad[i * per : (i + 1) * per]),
            "weight2": wt2,
            "bias2": b2,
        }
        for i in range(N_CORES)
    ]
    res = bass_utils.run_bass_kernel_spmd(
        nc, in_maps, core_ids=list(range(N_CORES)), trace=trace, **kw
    )
    out = np.concatenate([res.results[i]["out"] for i in range(N_CORES)], axis=0)
    return out, res


def kernel(image, weight, bias):
    out, _ = run_cores(image, weight, bias, trace=False)
    return out



# revision 8
# speedup vs baseline: 1.7748x; 1.7748x over previous
"""Trainium2 Bass kernel: 3x3 SAME conv (stride 1), NCHW fp32.

Problem: image [32, 64, 112, 112] * weight [64, 64, 3, 3] + bias [64]
Sharding: data-parallel over batch across 8 NeuronCores (4 images each).

Per-core strategy (v2: tap-pair K=128 matmuls):
  - Each 3x3 tap is a shift of the flat 114x114-padded image, so the conv
    for an output tile is 9 accumulating matmuls. Instead of K=64 (cin)
    matmuls, pack TWO taps into the K=128 contraction: partitions 0-63
    hold the image (per cin), partitions 64-127 hold a shifted copy, and
    one matmul contracts both taps at once. 5 passes replace 9:
      tile A (group1 = img shifted +1):   taps {(0,0),(0,1)}, {(1,0),(1,1)},
                                          {(2,0),(2,1)} and single (2,2)
      tile B (group1 = img shifted +114): taps {(0,2),(1,2)}
  - Output tile = 4 output rows x 114 cols (N=456 PSUM columns, garbage
    cols dropped on drain). Row-block pairs share one PSUM bank: even rb
    accumulates on psum partitions 0-63, odd rb on 64-127.
  - Operands bf16 (host-converted), fp32 PSUM accumulation.
  - Drain: DVE tensor_scalar_add (fused +bias) PSUM -> bf16 SBUF staging,
    then one 128-partition DMA per pair back to HBM (bf16 out, upcast on
    host) to keep total DMA below the PE time.
  - Image 0 startup: its first 7 row-block pairs use a 6-pass variant
    (the {(0,2),(1,2)} pair as two K=64 single-tap matmuls on tile A) so
    the PE never waits for B0's DMA.
"""

import numpy as np

import concourse.bass as bass
import concourse.mybir as mybir
import concourse.tile as tile
from concourse import bacc, bass_utils

N_CORES = 8
IMGS = 4  # images per core
CIN = 64
COUT = 64
H = 112
W = 112
HP = H + 2  # 114
WP = W + 2  # 114
F = HP * WP  # 12996 image floats per channel
CH_STRIDE = F  # per-channel stride in the flat HBM image
# each image slab: 1 lead zero, 64 packed channels, tail zero slack.
# image data lives at slab offset +1 so SBUF tiles (whose data sits at
# free offset 1, the lead cell feeding dropped garbage columns) can be
# filled edge-to-edge from initialized HBM bytes.
IMG_SLACK = 128
IMG_STRIDE = 1 + CIN * CH_STRIDE + IMG_SLACK - 1
AF = 1 + F + 3  # SBUF tile free size: lead garbage cell + img + tail pad
NMM = 4 * WP  # 456 matmul free size (4 output rows)
NOUT = 4 * W  # 448 valid outputs per rb per channel
NRB = H // 4  # 28 row blocks per image
NPAIR = NRB // 2  # 14 row-block pairs
SIX_PASS_PAIRS = 7  # image-0 pairs that avoid tile B

F32 = mybir.dt.float32
BF16 = mybir.dt.bfloat16


def _ap(ap_obj, offset, dims):
    """Manual AP on the same tensor handle; dims = [[step, count], ...]."""
    return bass.AP(tensor=ap_obj.tensor, offset=offset, ap=dims)


def build_nc(n_imgs=IMGS, mm_dtype=BF16):
    nc = bacc.Bacc(
        "TRN2",
        target_bir_lowering=False,
        debug=False,
        num_devices=N_CORES,
    )
    img_d = nc.dram_tensor("image_flat", [n_imgs * IMG_STRIDE], mm_dtype, kind="ExternalInput")
    wt_d = nc.dram_tensor("weight2", [128, 6 * COUT], mm_dtype, kind="ExternalInput")
    bias_d = nc.dram_tensor("bias2", [128, 1], F32, kind="ExternalInput")
    out_d = nc.dram_tensor("out", [n_imgs, COUT, H, W], mm_dtype, kind="ExternalOutput")

    img_ap = img_d.ap()
    out_ap = out_d.ap()

    with tile.TileContext(nc) as tc:
        with (
            tc.tile_pool(name="imga", bufs=2) as a_pool,
            tc.tile_pool(name="imgb", bufs=2) as b_pool,
            tc.tile_pool(name="wt", bufs=1) as wt_pool,
            tc.tile_pool(name="bias", bufs=1) as bias_pool,
            tc.tile_pool(name="stage", bufs=10) as stage_pool,
            tc.tile_pool(name="psum", bufs=8, space="PSUM") as psum_pool,
        ):
            wt_t = wt_pool.tile([128, 6 * COUT], mm_dtype)
            nc.sync.dma_start(wt_t[:], wt_d.ap()[:])
            bias_t = bias_pool.tile([128, 1], F32)
            nc.sync.dma_start(bias_t[:], bias_d.ap()[:])

            HLEN = AF // 2  # 6500, split loads so the serialized DMA
            # resource frees up between halves (drain DMAs interleave)

            def load_img(n):
                base = n * IMG_STRIDE
                # tile A: partitions 0-63 = img at free offset 1 (per cin),
                # partitions 64-127 = img shifted +1. Whole tile filled:
                # cell 0 / tails read lead-zero or neighbor-channel bytes,
                # which only ever feed dropped garbage columns.
                a_t = a_pool.tile([128, AF], mm_dtype)
                for h in range(2):
                    nc.sync.dma_start(
                        a_t[:, h * HLEN : (h + 1) * HLEN],
                        _ap(img_ap, base + h * HLEN,
                            [[1, 2], [CH_STRIDE, CIN], [1, HLEN]]),
                    )
                # tile B: partitions 0-63 = img, 64-127 = img shifted +114.
                # (chan 63's +114 tail reads the zero inter-image slack.)
                b_t = b_pool.tile([128, AF], mm_dtype)
                for h in range(2):
                    nc.sync.dma_start(
                        b_t[:, h * HLEN : (h + 1) * HLEN],
                        _ap(img_ap, base + h * HLEN,
                            [[114, 2], [CH_STRIDE, CIN], [1, HLEN]]),
                    )
                return a_t, b_t

            tiles = load_img(0)
            for n in range(n_imgs):
                a_t, b_t = tiles
                if n + 1 < n_imgs:
                    # prefetch next image; issued before this image's drains
                    # so out-DMAs precede the (gated) reload in queue order
                    tiles = load_img(n + 1)
                for pair in range(NPAIR):
                    six_pass = n == 0 and pair < SIX_PASS_PAIRS
                    ps = psum_pool.tile([128, NMM], F32)
                    for rb, pb in ((2 * pair, 0), (2 * pair + 1, 64)):
                        dst = ps[pb : pb + 64, :]
                        o = 4 * rb * WP
                        # kh-row passes: group0 tap (kh,0), group1 tap (kh,1)
                        for kh in range(3):
                            nc.tensor.matmul(
                                dst,
                                wt_t[:, kh * 64 : (kh + 1) * 64],
                                a_t[:, o + kh * WP : o + kh * WP + NMM],
                                start=(kh == 0), stop=False,
                                skip_group_check=True,
                            )
                        if six_pass:
                            # taps (0,2) and (1,2) as K=64 singles on A.
                            # both use base-partition-0 operands: the device
                            # rejects base-64 K=64 matmuls mixed into a
                            # K=128-started psum group, so W(1,2) is
                            # duplicated on rows 0-63 at cols 320:384.
                            nc.tensor.matmul(
                                dst,
                                wt_t[0:64, 192:256],
                                a_t[0:64, o + 2 : o + 2 + NMM],
                                start=False, stop=False,
                                skip_group_check=True,
                            )
                            nc.tensor.matmul(
                                dst,
                                wt_t[0:64, 320:384],
                                a_t[0:64, o + WP + 2 : o + WP + 2 + NMM],
                                start=False, stop=False,
                                skip_group_check=True,
                            )
                        else:
                            # group0 tap (0,2), group1 tap (1,2) via B
                            nc.tensor.matmul(
                                dst,
                                wt_t[:, 192:256],
                                b_t[:, o + 2 : o + 2 + NMM],
                                start=False, stop=False,
                                skip_group_check=True,
                            )
                        # single tap (2,2), K=64 on A group0
                        nc.tensor.matmul(
                            dst,
                            wt_t[0:64, 256:320],
                            a_t[0:64, o + 2 * WP + 2 : o + 2 * WP + 2 + NMM],
                            start=False, stop=True,
                            skip_group_check=True,
                        )

                    # drain + bias: psum[:, 4x(1..113)] -> bf16 stage[:, 448].
                    # partitions = [rb even couts | rb odd couts] -> one
                    # contiguous-run DMA (8 rows per chan, 2x4x112).
                    stg = stage_pool.tile([128, NOUT], mm_dtype)
                    src_ps = ps[:].rearrange("p (r c) -> p r c", r=4)[:, :, 1 : 1 + W]
                    dst_st = stg[:].rearrange("p (r c) -> p r c", r=4)
                    nc.vector.tensor_scalar_add(dst_st, src_ps, bias_t[:])
                    obase = n * COUT * H * W + 8 * pair * W
                    nc.sync.dma_start(
                        _ap(out_ap, obase, [[4 * W, 2], [H * W, COUT], [1, NOUT]]),
                        stg[:],
                    )

    nc.compile()
    return nc


_NC_CACHE = {}


def _get_nc(n_imgs=IMGS):
    if n_imgs not in _NC_CACHE:
        _NC_CACHE[n_imgs] = build_nc(n_imgs)
    return _NC_CACHE[n_imgs]


def _prep_inputs(image, weight, bias):
    import ml_dtypes

    image = np.asarray(image, dtype=np.float32)
    weight = np.asarray(weight, dtype=np.float32)
    bias = np.asarray(bias, dtype=np.float32)
    n = image.shape[0]
    bf16 = ml_dtypes.bfloat16
    img_flat = np.zeros((n, IMG_STRIDE), bf16)
    pad = np.zeros((n, CIN, HP, WP), bf16)
    pad[:, :, 1 : 1 + H, 1 : 1 + W] = image.astype(bf16)
    img_flat[:, 1 : 1 + CIN * CH_STRIDE] = pad.reshape(n, CIN * CH_STRIDE)
    # lhsT per pass: [cin(K), cout(M)]; tap pairs stacked on partition halves
    wt4 = weight.astype(np.float32).transpose(1, 2, 3, 0)  # [cin, kh, kw, cout]
    wt2 = np.zeros((128, 6 * COUT), np.float32)
    # passes 0-2: taps (kh,0) on rows 0-63, (kh,1) on rows 64-127
    for kh in range(3):
        wt2[0:64, kh * 64 : (kh + 1) * 64] = wt4[:, kh, 0, :]
        wt2[64:128, kh * 64 : (kh + 1) * 64] = wt4[:, kh, 1, :]
    # pass 3: taps (0,2) / (1,2); pass 4: tap (2,2) on rows 0-63
    wt2[0:64, 192:256] = wt4[:, 0, 2, :]
    wt2[64:128, 192:256] = wt4[:, 1, 2, :]
    wt2[0:64, 256:320] = wt4[:, 2, 2, :]
    # warmup six-pass singles: W(1,2) duplicated on rows 0-63
    wt2[0:64, 320:384] = wt4[:, 1, 2, :]
    b2 = np.concatenate([bias, bias]).reshape(128, 1)
    return img_flat.reshape(-1), wt2.astype(bf16), b2


def run_cores(image, weight, bias, trace=False, **kw):
    """Shard over 8 cores, run, return (full_output, BassKernelResults)."""
    img_flat, wt2, b2 = _prep_inputs(image, weight, bias)
    n = image.shape[0]
    per = n // N_CORES
    assert per * N_CORES == n
    nc = _get_nc(per)
    img_flat = img_flat.reshape(N_CORES, per * IMG_STRIDE)
    in_maps = [
        {
            "image_flat": np.ascontiguousarray(img_flat[i]),
            "weight2": wt2,
            "bias2": b2,
        }
        for i in range(N_CORES)
    ]
    res = bass_utils.run_bass_kernel_spmd(
        nc, in_maps, core_ids=list(range(N_CORES)), trace=trace, **kw
    )
    out = np.concatenate(
        [np.asarray(res.results[i]["out"]) for i in range(N_CORES)], axis=0
    ).astype(np.float32)
    return out, res


def kernel(image, weight, bias):
    out, _ = run_cores(image, weight, bias, trace=False)
    return out
